# revision 9
# baseline (speedup 1.0000x reference)
"""DAG-constraint layer kernel for Trainium2 (8 NeuronCores, data parallel).

The reference (p = sigmoid(x); iterative min/max projection over the
chain+skip DAG on N=32 nodes) collapses to a per-row prefix-min:

    out[b, j] = min_{k <= j} sigmoid(x[b, k]) = sigmoid(cummin(x, axis=1))

(verified bitwise against the reference by an earlier session).

Measured TRN2 rates drive the design (all fp16; tolerance is 2e-2 rel and
fp16 end-to-end lands ~2.4e-3):
  - hardware scan (TensorTensorScanArith): 2.09 ns/free-elem, dtype-blind
  - tensor_tensor min, packed fp16 (2x mode): 0.52 ns/free-elem
  - ACT sigmoid: 0.833 ns/free-elem + 294 ns/instr, in-place free
  - DMA rings throttle to ~190 GB/s/ring with all 8 cores running
so the serial scan runs over only every 8th column (block minima), and
everything else is packed-fp16 elementwise work:

  1. Host permutes each partition-chunk of G rows CLASS-MAJOR:
     [G, 32] -> [G, 4, 8] -> [8(class c), G, 4(block q)], so column
     j = 8q + c lands in contiguous class block c.  Every device operand
     is then a fully contiguous packed-fp16 [P, Fc] slab.
  2. W-chain  W_c = min(W_{c-1}, p_c)  (7 TT mins): within-block prefix
     minima; W_7 = block minimum.
  3. Segmented EXCLUSIVE scan over W_7 shifted one slot left, with a mask
     holding +BIG at every block-q==0 slot: out = max(min(d0, state), d1).
     The +BIG resets the running state at each row start AND emits the
     exclusive identity, so no repair passes exist anywhere.
  4. m_c = min(S, W_c) (8 independent TT mins) = final prefix minima.

Sigmoid placement is HYBRID to kill both serial tails (sigmoid commutes
with min): tile A (processed first on DVE) takes sigmoid AFTER the min
machinery - sigma(m_A) trails mid-pipeline on ACT; tile B takes sigmoid
BEFORE (sigma(x_B) runs upstream on ACT during the DVE's tile-A phase),
so tile B's outputs DMA straight out after the last DVE op with no
trailing sigmoid.  Inputs are split across all three DMA rings (sync,
gpsimd/SWDGE, scalar) so the ~190 GB/s/ring fill phase never starves the
pipeline; outputs ride sync (even chunks) and gpsimd (odd chunks).
"""

import os
import subprocess
import sys
import tempfile
from contextlib import ExitStack

import numpy as np

import concourse.bass as bass
import concourse.mybir as mybir
from concourse.bass_utils import run_bass_kernel_spmd

N_CORES = 8
B_TOTAL = 524288
N_NODES = 32
ROWS_PER_CORE = B_TOTAL // N_CORES  # 65536
P = 128
BLK = 8                             # columns per scan block (class count)
NB = N_NODES // BLK                 # blocks per row = 4
FSIZES = [8192, 8192]               # [tile A, tile B] free elems/partition
NT = len(FSIZES)
POS_BIG = 60000.0                   # fp16-representable poison
NEG_BIG = -60000.0

assert sum(FSIZES) * P == ROWS_PER_CORE * N_NODES
assert all(f % N_NODES == 0 for f in FSIZES)
FC = [f // BLK for f in FSIZES]


def _build() -> bass.Bass:
    nc = bass.Bass()
    f16 = mybir.dt.float16
    mn = mybir.AluOpType.min
    mx = mybir.AluOpType.max
    x = nc.declare_dram_parameter("x", [ROWS_PER_CORE, N_NODES], f16, isOutput=False)
    y = nc.declare_dram_parameter("y", [ROWS_PER_CORE, N_NODES], f16, isOutput=True)
    xf = x[:].flatten()
    yf = y[:].flatten()
    offs = [0]
    for fsz in FSIZES:
        offs.append(offs[-1] + P * fsz)

    def _dram(flat, t):
        return flat[offs[t] : offs[t + 1]].rearrange("(p f) -> p f", p=P)

    def _cchunk(ap_2d, t, k):  # class-pair chunk k (classes 2k, 2k+1)
        fc = FC[t]
        return ap_2d[:, 2 * k * fc : 2 * (k + 1) * fc]

    A, B = 0, 1  # tile roles: A = sigmoid-after, B = sigmoid-first

    with ExitStack() as es:
        ec = es.enter_context
        xts = [ec(nc.sbuf_tensor(f"xt{t}", [P, FSIZES[t]], f16)) for t in range(NT)]
        wts = [ec(nc.sbuf_tensor(f"wt{t}", [P, 7 * FC[t]], f16)) for t in range(NT)]
        sts = [ec(nc.sbuf_tensor(f"st{t}", [P, FC[t]], f16)) for t in range(NT)]
        mts = [ec(nc.sbuf_tensor(f"mt{t}", [P, FSIZES[t]], f16)) for t in range(NT)]
        mask = ec(nc.sbuf_tensor("mask", [P, max(FC)], f16))
        warm = ec(nc.sbuf_tensor("act_warm", [P, 1], f16))
        in_a = [ec(nc.semaphore(f"ina{k}")) for k in range(4)]
        in_b = [ec(nc.semaphore(f"inb{k}")) for k in range(4)]
        sig = ec(nc.semaphore("sig"))    # ACT: sigma_x(B) chunks 1-4, sigma_m(A) 5-8
        msem = ec(nc.semaphore("msem"))  # DVE: m quarter-chunks, A 1-4 then B 5-8
        osy = ec(nc.semaphore("osy"))
        ogp = ec(nc.semaphore("ogp"))

        with nc.Block(no_gpsimd_drain=True) as block:

            @block.sync
            def _(sync):
                # ALL of tile A's input on the sync ring (HW DGE: lowest
                # first-chunk latency; these gate the DVE pipeline start).
                for k in range(4):
                    sync.dma_start(
                        out=_cchunk(xts[A][:], A, k), in_=_cchunk(_dram(xf, A), A, k)
                    ).then_inc(in_a[k], 16)
                # B-outs (tail-critical) also on the sync ring
                for k in range(4):
                    sync.wait_ge(msem, 5 + k)
                    sync.dma_start(
                        out=_cchunk(_dram(yf, B), B, k), in_=_cchunk(mts[B][:], B, k)
                    ).then_inc(osy, 16)
                sync.wait_ge(osy, 16 * 4)

            @block.gpsimd
            def _(gp):
                # A-outs are mid-pipeline and latency-tolerant: SWDGE ring
                for k in range(4):
                    gp.wait_ge(sig, 5 + k)
                    gp.dma_start(
                        out=_cchunk(_dram(yf, A), A, k), in_=_cchunk(mts[A][:], A, k)
                    ).then_inc(ogp, 16)
                gp.wait_ge(ogp, 16 * 4)

            @block.scalar
            def _(scalar):
                # tile B input on the scalar ring (dispatched before any
                # activation so the ring fills during the warm-up)
                for k in range(4):
                    scalar.dma_start(
                        out=_cchunk(xts[B][:], B, k), in_=_cchunk(_dram(xf, B), B, k)
                    ).then_inc(in_b[k], 16)
                scalar.activation(
                    out=warm[:], in_=warm[:],
                    func=mybir.ActivationFunctionType.Sigmoid,
                )
                # sigma_x over tile B input chunks (upstream of DVE)
                for k in range(4):
                    scalar.wait_ge(in_b[k], 16)
                    scalar.activation(
                        out=_cchunk(xts[B][:], B, k), in_=_cchunk(xts[B][:], B, k),
                        func=mybir.ActivationFunctionType.Sigmoid,
                    ).then_inc(sig, 1)
                # sigma_m over tile A m-chunks (downstream, mid-pipeline)
                for k in range(4):
                    scalar.wait_ge(msem, k + 1)
                    scalar.activation(
                        out=_cchunk(mts[A][:], A, k), in_=_cchunk(mts[A][:], A, k),
                        func=mybir.ActivationFunctionType.Sigmoid,
                    ).then_inc(sig, 1)

            @block.vector
            def _(vector):
                # mask: -BIG everywhere, +BIG at block-q==0 slots (period NB)
                vector.memset(mask[:], NEG_BIG)
                vector.memset(
                    mask[:].rearrange("p (g q) -> p g q", q=NB)[:, :, 0], POS_BIG
                )

                def tile(t, gate_sems=None, gate_sig=False):
                    fc = FC[t]
                    xt, wt, st, mt = xts[t], wts[t], sts[t], mts[t]

                    def cls(c):
                        return xt[:, c * fc : (c + 1) * fc]

                    def wc(c):
                        return wt[:, (c - 1) * fc : c * fc]

                    def gate(k):
                        if gate_sems is not None:
                            vector.wait_ge(gate_sems[k], 16)
                        if gate_sig:
                            vector.wait_ge(sig, k + 1)

                    gate(0)
                    vector.tensor_tensor(out=wc(1), in0=cls(0), in1=cls(1), op=mn)
                    gate(1)
                    vector.tensor_tensor(out=wc(2), in0=wc(1), in1=cls(2), op=mn)
                    vector.tensor_tensor(out=wc(3), in0=wc(2), in1=cls(3), op=mn)
                    gate(2)
                    vector.tensor_tensor(out=wc(4), in0=wc(3), in1=cls(4), op=mn)
                    vector.tensor_tensor(out=wc(5), in0=wc(4), in1=cls(5), op=mn)
                    gate(3)
                    vector.tensor_tensor(out=wc(6), in0=wc(5), in1=cls(6), op=mn)
                    vector.tensor_tensor(out=wc(7), in0=wc(6), in1=cls(7), op=mn)
                    # exclusive segmented scan over shifted W_7
                    vector.tensor_tensor_scan(
                        out=st[:],
                        data0=wt[:, 6 * fc - 1 : 7 * fc - 1],
                        data1=mask[:, :fc],
                        initial=POS_BIG,
                        op0=mn,
                        op1=mx,
                    )
                    # m_c = min(S, W_c); quarter-tile msem increments
                    vector.tensor_tensor(out=mt[:, :fc], in0=st[:], in1=cls(0), op=mn)
                    for c in range(1, BLK):
                        op = vector.tensor_tensor(
                            out=mt[:, c * fc : (c + 1) * fc],
                            in0=st[:], in1=wc(c), op=mn,
                        )
                        if c % 2 == 1:
                            op.then_inc(msem, 1)

                tile(A, gate_sems=in_a)   # raw input, sigmoid comes after
                tile(B, gate_sig=True)    # sigmoid'd input, outputs final

    return nc


def _permute_in(x8: np.ndarray) -> np.ndarray:
    """[8, ROWS, 32] fp16 -> class-major flat [8, ROWS*32]."""
    parts = []
    row0 = 0
    for fsz in FSIZES:
        g = fsz // N_NODES
        band = x8[:, row0 : row0 + P * g].reshape(N_CORES, P, g, NB, BLK)
        parts.append(band.transpose(0, 1, 4, 2, 3).reshape(N_CORES, P * fsz))
        row0 += P * g
    return np.concatenate(parts, axis=1)


def _unpermute_out(yp: np.ndarray) -> np.ndarray:
    """[8, ROWS*32] class-major flat -> [8, ROWS, 32]."""
    outs = []
    col0 = 0
    for fsz in FSIZES:
        g = fsz // N_NODES
        band = yp[:, col0 : col0 + P * fsz].reshape(N_CORES, P, BLK, g, NB)
        outs.append(band.transpose(0, 1, 3, 4, 2).reshape(N_CORES, P * g, N_NODES))
        col0 += P * fsz
    return np.concatenate(outs, axis=1)


def _run(x: np.ndarray, trace: bool = False):
    x = np.ascontiguousarray(np.asarray(x), dtype=np.float16)
    assert x.shape == (B_TOTAL, N_NODES), x.shape
    nc = _build()
    xp = _permute_in(x.reshape(N_CORES, ROWS_PER_CORE, N_NODES))
    in_maps = [
        {"x": xp[i].reshape(ROWS_PER_CORE, N_NODES)} for i in range(N_CORES)
    ]
    res = run_bass_kernel_spmd(nc, in_maps, list(range(N_CORES)), trace=trace)
    yp = np.stack(
        [res.results[i]["y"].reshape(-1) for i in range(N_CORES)], axis=0
    )
    out = _unpermute_out(yp).reshape(B_TOTAL, N_NODES).astype(np.float32)
    return out, res


def _trn_devices_visible() -> bool:
    try:
        import jax

        return sum(1 for d in jax.devices() if d.platform != "cpu") >= N_CORES
    except Exception:
        return False


def _run_in_subprocess(x: np.ndarray) -> np.ndarray:
    with tempfile.TemporaryDirectory() as td:
        xin = os.path.join(td, "x.npy")
        xout = os.path.join(td, "y.npy")
        np.save(xin, x)
        env = dict(os.environ)
        for k in ("JAX_PLATFORMS", "JAX_PLATFORM_NAME"):
            env.pop(k, None)
        subprocess.run(
            [sys.executable, os.path.abspath(__file__), xin, xout],
            check=True,
            env=env,
        )
        return np.load(xout)


def kernel(x, children=None, child_mask=None, parents=None, parent_mask=None,
           topo=None, **_unused):
    x = np.ascontiguousarray(np.asarray(x), dtype=np.float32)
    if _trn_devices_visible():
        out, _ = _run(x)
        return out
    return _run_in_subprocess(x)


if __name__ == "__main__":
    _x = np.load(sys.argv[1])
    _out, _ = _run(_x)
    np.save(sys.argv[2], _out)


# revision 13
# speedup vs baseline: 1.0261x; 1.0261x over previous
"""DAG-constraint layer kernel for Trainium2 (8 NeuronCores, data parallel).

The reference (p = sigmoid(x); iterative min/max projection over the
chain+skip DAG on N=32 nodes) collapses to a per-row prefix-min:

    out[b, j] = min_{k <= j} sigmoid(x[b, k]) = sigmoid(cummin(x, axis=1))

(verified bitwise against the reference by an earlier session).

Measured TRN2 rates drive the design (all fp16; tolerance is 2e-2 rel and
fp16 end-to-end lands ~2.4e-3):
  - hardware scan (TensorTensorScanArith): 2.09 ns/free-elem, dtype-blind
  - tensor_tensor min, packed fp16 (2x mode): 0.52 ns/free-elem
  - ACT sigmoid: 0.833 ns/free-elem + 294 ns/instr, in-place free
  - DMA rings throttle to ~190 GB/s/ring with all 8 cores running
so the serial scan runs over only every 8th column (block minima), and
everything else is packed-fp16 elementwise work:

  1. Host permutes each partition-chunk of G rows CLASS-MAJOR:
     [G, 32] -> [G, 4, 8] -> [8(class c), G, 4(block q)], so column
     j = 8q + c lands in contiguous class block c.  Every device operand
     is then a fully contiguous packed-fp16 [P, Fc] slab.
  2. W-chain  W_c = min(W_{c-1}, p_c)  (7 TT mins): within-block prefix
     minima; W_7 = block minimum.
  3. Segmented EXCLUSIVE scan over W_7 shifted one slot left, with a mask
     holding +BIG at every block-q==0 slot: out = max(min(d0, state), d1).
     The +BIG resets the running state at each row start AND emits the
     exclusive identity, so no repair passes exist anywhere.
  4. m_c = min(S, W_c) (8 independent TT mins) = final prefix minima.

Sigmoid placement is HYBRID to kill both serial tails (sigmoid commutes
with min): tile A (processed first on DVE) takes sigmoid AFTER the min
machinery - sigma(m_A) trails mid-pipeline on ACT; tile B takes sigmoid
BEFORE (sigma(x_B) runs upstream on ACT during the DVE's tile-A phase),
so tile B's outputs DMA straight out after the last DVE op with no
trailing sigmoid.  Inputs are split across all three DMA rings (sync,
gpsimd/SWDGE, scalar) so the ~190 GB/s/ring fill phase never starves the
pipeline; outputs ride sync (even chunks) and gpsimd (odd chunks).
"""

import os
import subprocess
import sys
import tempfile
from contextlib import ExitStack

import numpy as np

import concourse.bass as bass
import concourse.mybir as mybir
from concourse.bass_utils import run_bass_kernel_spmd

N_CORES = 8
B_TOTAL = 524288
N_NODES = 32
ROWS_PER_CORE = B_TOTAL // N_CORES  # 65536
P = 128
BLK = 8                             # columns per scan block (class count)
NB = N_NODES // BLK                 # blocks per row = 4
FSIZES = [8192, 8192]               # [tile A, tile B] free elems/partition
NT = len(FSIZES)
POS_BIG = 60000.0                   # fp16-representable poison
NEG_BIG = -60000.0

assert sum(FSIZES) * P == ROWS_PER_CORE * N_NODES
assert all(f % N_NODES == 0 for f in FSIZES)
FC = [f // BLK for f in FSIZES]


def _build() -> bass.Bass:
    nc = bass.Bass()
    f16 = mybir.dt.float16
    mn = mybir.AluOpType.min
    mx = mybir.AluOpType.max
    x = nc.declare_dram_parameter("x", [ROWS_PER_CORE, N_NODES], f16, isOutput=False)
    y = nc.declare_dram_parameter("y", [ROWS_PER_CORE, N_NODES], f16, isOutput=True)
    xf = x[:].flatten()
    yf = y[:].flatten()
    offs = [0]
    for fsz in FSIZES:
        offs.append(offs[-1] + P * fsz)

    def _dram(flat, t):
        return flat[offs[t] : offs[t + 1]].rearrange("(p f) -> p f", p=P)

    def _cchunk(ap_2d, t, k):  # class-pair chunk k (classes 2k, 2k+1)
        fc = FC[t]
        return ap_2d[:, 2 * k * fc : 2 * (k + 1) * fc]

    A, B = 0, 1  # tile roles: A = sigmoid-after, B = sigmoid-first

    with ExitStack() as es:
        ec = es.enter_context
        xts = [ec(nc.sbuf_tensor(f"xt{t}", [P, FSIZES[t]], f16)) for t in range(NT)]
        wts = [ec(nc.sbuf_tensor(f"wt{t}", [P, 7 * FC[t]], f16)) for t in range(NT)]
        sts = [ec(nc.sbuf_tensor(f"st{t}", [P, FC[t]], f16)) for t in range(NT)]
        mts = [ec(nc.sbuf_tensor(f"mt{t}", [P, FSIZES[t]], f16)) for t in range(NT)]
        mask = ec(nc.sbuf_tensor("mask", [P, max(FC)], f16))
        warm = ec(nc.sbuf_tensor("act_warm", [P, 1], f16))
        in_a = [ec(nc.semaphore(f"ina{k}")) for k in range(5)]
        in_b = [ec(nc.semaphore(f"inb{k}")) for k in range(4)]
        sig = ec(nc.semaphore("sig"))    # ACT: sigma_x(B) chunks 1-4, sigma_m(A) 5-6
        msem = ec(nc.semaphore("msem"))  # DVE: 1 = m(A) done; 2-4 = m(B) stages
        osy = ec(nc.semaphore("osy"))
        ogp = ec(nc.semaphore("ogp"))

        with nc.Block(no_gpsimd_drain=True) as block:

            @block.sync
            def _(sync):
                # ALL of tile A's input on the sync ring (HW DGE: lowest
                # first-chunk latency; these gate the DVE pipeline start).
                # The first class-pair is split row-wise so the very first
                # 256 KiB lands ~1.4us earlier and W1a can start on it.
                fc = FC[A]
                half = fc // 2
                xa, da = xts[A][:], _dram(xf, A)
                for k, (lo, hi) in enumerate(((0, half), (half, fc))):
                    src = da.rearrange("p (c f) -> p c f", c=BLK)[:, 0:2, lo:hi]
                    dst = xa.rearrange("p (c f) -> p c f", c=BLK)[:, 0:2, lo:hi]
                    sync.dma_start(out=dst, in_=src).then_inc(in_a[k], 16)
                for k in range(1, 4):
                    sync.dma_start(
                        out=_cchunk(xts[A][:], A, k), in_=_cchunk(_dram(xf, A), A, k)
                    ).then_inc(in_a[k + 1], 16)
                # B-outs (tail-critical) also on the sync ring, staged so the
                # final transfer is a single class (256 KiB)
                fcb = FC[B]
                sync.wait_ge(msem, 2)
                sync.dma_start(
                    out=_dram(yf, B)[:, : 6 * fcb], in_=mts[B][:, : 6 * fcb]
                ).then_inc(osy, 16)
                sync.wait_ge(msem, 3)
                sync.dma_start(
                    out=_dram(yf, B)[:, 6 * fcb : 7 * fcb],
                    in_=mts[B][:, 6 * fcb : 7 * fcb],
                ).then_inc(osy, 16)
                sync.wait_ge(msem, 4)
                sync.dma_start(
                    out=_dram(yf, B)[:, 7 * fcb :], in_=mts[B][:, 7 * fcb :]
                ).then_inc(osy, 16)
                sync.wait_ge(osy, 16 * 3)

            @block.gpsimd
            def _(gp):
                # A-outs are mid-pipeline and latency-tolerant: SWDGE ring
                for k in range(2):
                    gp.wait_ge(sig, 5 + k)
                    gp.dma_start(
                        out=_dram(yf, A)[:, 4 * k * FC[A] : 4 * (k + 1) * FC[A]],
                        in_=mts[A][:, 4 * k * FC[A] : 4 * (k + 1) * FC[A]],
                    ).then_inc(ogp, 16)
                gp.wait_ge(ogp, 16 * 2)

            @block.scalar
            def _(scalar):
                # tile B input on the scalar ring (dispatched before any
                # activation so the ring fills during the warm-up)
                for k in range(4):
                    scalar.dma_start(
                        out=_cchunk(xts[B][:], B, k), in_=_cchunk(_dram(xf, B), B, k)
                    ).then_inc(in_b[k], 16)
                scalar.activation(
                    out=warm[:], in_=warm[:],
                    func=mybir.ActivationFunctionType.Sigmoid,
                )
                # sigma_x over tile B input chunks (upstream of DVE)
                for k in range(4):
                    scalar.wait_ge(in_b[k], 16)
                    scalar.activation(
                        out=_cchunk(xts[B][:], B, k), in_=_cchunk(xts[B][:], B, k),
                        func=mybir.ActivationFunctionType.Sigmoid,
                    ).then_inc(sig, 1)
                # sigma_m over tile A m-halves (downstream, mid-pipeline)
                scalar.wait_ge(msem, 1)
                for k in range(2):
                    scalar.activation(
                        out=mts[A][:, 4 * k * FC[A] : 4 * (k + 1) * FC[A]],
                        in_=mts[A][:, 4 * k * FC[A] : 4 * (k + 1) * FC[A]],
                        func=mybir.ActivationFunctionType.Sigmoid,
                    ).then_inc(sig, 1)

            @block.vector
            def _(vector):
                # mask: -BIG everywhere, +BIG at block-q==0 slots (period NB)
                vector.memset(mask[:], NEG_BIG)
                vector.memset(
                    mask[:].rearrange("p (g q) -> p g q", q=NB)[:, :, 0], POS_BIG
                )

                def bcast_m(vector_, st_, wt_, mt_, fc, c_lo, c_hi):
                    """m_c = min(S, W_c) for c in [c_lo, c_hi] as ONE op:
                    S broadcast with a stride-0 middle dim."""
                    ncl = c_hi - c_lo + 1
                    s1 = st_[:].rearrange("p (o f) -> p o f", o=1)
                    wv = wt_[:, (c_lo - 1) * fc : c_hi * fc].rearrange(
                        "p (o f) -> p o f", o=ncl
                    )
                    sB, wB = bass.broadcast_tensor_aps(s1, wv)
                    out = mt_[:, c_lo * fc : (c_hi + 1) * fc].rearrange(
                        "p (o f) -> p o f", o=ncl
                    )
                    return vector_.tensor_tensor(out=out, in0=sB, in1=wB, op=mn)

                def tile(t, gate_sems=None, gate_sig=False):
                    fc = FC[t]
                    xt, wt, st, mt = xts[t], wts[t], sts[t], mts[t]

                    def cls(c):
                        return xt[:, c * fc : (c + 1) * fc]

                    def wc(c):
                        return wt[:, (c - 1) * fc : c * fc]

                    def gate(k):
                        if gate_sems is not None:
                            vector.wait_ge(gate_sems[k], 16)
                        if gate_sig:
                            vector.wait_ge(sig, k + 1)

                    if gate_sems is not None:
                        # first class-pair arrives as two row-half DMAs
                        half = fc // 2
                        vector.wait_ge(gate_sems[0], 16)
                        vector.tensor_tensor(
                            out=wt[:, :half], in0=xt[:, :half],
                            in1=xt[:, fc : fc + half], op=mn,
                        )
                        vector.wait_ge(gate_sems[1], 16)
                        vector.tensor_tensor(
                            out=wt[:, half:fc], in0=xt[:, half:fc],
                            in1=xt[:, fc + half : 2 * fc], op=mn,
                        )
                        gates = [None, gate_sems[2], gate_sems[3], gate_sems[4]]

                        def gate(k):  # noqa: F811
                            if gates[k] is not None:
                                vector.wait_ge(gates[k], 16)
                    else:
                        gate(0)
                        vector.tensor_tensor(out=wc(1), in0=cls(0), in1=cls(1), op=mn)
                    gate(1)
                    vector.tensor_tensor(out=wc(2), in0=wc(1), in1=cls(2), op=mn)
                    vector.tensor_tensor(out=wc(3), in0=wc(2), in1=cls(3), op=mn)
                    gate(2)
                    vector.tensor_tensor(out=wc(4), in0=wc(3), in1=cls(4), op=mn)
                    vector.tensor_tensor(out=wc(5), in0=wc(4), in1=cls(5), op=mn)
                    gate(3)
                    vector.tensor_tensor(out=wc(6), in0=wc(5), in1=cls(6), op=mn)
                    vector.tensor_tensor(out=wc(7), in0=wc(6), in1=cls(7), op=mn)
                    # exclusive segmented scan over shifted W_7
                    vector.tensor_tensor_scan(
                        out=st[:],
                        data0=wt[:, 6 * fc - 1 : 7 * fc - 1],
                        data1=mask[:, :fc],
                        initial=POS_BIG,
                        op0=mn,
                        op1=mx,
                    )
                    vector.tensor_tensor(out=mt[:, :fc], in0=st[:], in1=cls(0), op=mn)
                    if t == A:
                        # one fused broadcast for c=1..7; sigma_m gates on it
                        bcast_m(vector, st, wt, mt, fc, 1, 7).then_inc(msem, 1)
                    else:
                        # staged: c1-5 fused (big out chunk), then c6, c7
                        # singles so the final transfers are 256 KiB each
                        bcast_m(vector, st, wt, mt, fc, 1, 5).then_inc(msem, 1)
                        vector.tensor_tensor(
                            out=mt[:, 6 * fc : 7 * fc], in0=st[:], in1=wc(6), op=mn
                        ).then_inc(msem, 1)
                        vector.tensor_tensor(
                            out=mt[:, 7 * fc :], in0=st[:], in1=wc(7), op=mn
                        ).then_inc(msem, 1)

                tile(A, gate_sems=in_a)   # raw input, sigmoid comes after
                tile(B, gate_sig=True)    # sigmoid'd input, outputs final

    return nc


def _permute_in(x8: np.ndarray) -> np.ndarray:
    """[8, ROWS, 32] fp16 -> class-major flat [8, ROWS*32]."""
    parts = []
    row0 = 0
    for fsz in FSIZES:
        g = fsz // N_NODES
        band = x8[:, row0 : row0 + P * g].reshape(N_CORES, P, g, NB, BLK)
        parts.append(band.transpose(0, 1, 4, 2, 3).reshape(N_CORES, P * fsz))
        row0 += P * g
    return np.concatenate(parts, axis=1)


def _unpermute_out(yp: np.ndarray) -> np.ndarray:
    """[8, ROWS*32] class-major flat -> [8, ROWS, 32]."""
    outs = []
    col0 = 0
    for fsz in FSIZES:
        g = fsz // N_NODES
        band = yp[:, col0 : col0 + P * fsz].reshape(N_CORES, P, BLK, g, NB)
        outs.append(band.transpose(0, 1, 3, 4, 2).reshape(N_CORES, P * g, N_NODES))
        col0 += P * fsz
    return np.concatenate(outs, axis=1)


def _run(x: np.ndarray, trace: bool = False):
    x = np.ascontiguousarray(np.asarray(x), dtype=np.float16)
    assert x.shape == (B_TOTAL, N_NODES), x.shape
    nc = _build()
    xp = _permute_in(x.reshape(N_CORES, ROWS_PER_CORE, N_NODES))
    in_maps = [
        {"x": xp[i].reshape(ROWS_PER_CORE, N_NODES)} for i in range(N_CORES)
    ]
    res = run_bass_kernel_spmd(nc, in_maps, list(range(N_CORES)), trace=trace)
    yp = np.stack(
        [res.results[i]["y"].reshape(-1) for i in range(N_CORES)], axis=0
    )
    out = _unpermute_out(yp).reshape(B_TOTAL, N_NODES).astype(np.float32)
    return out, res


def _trn_devices_visible() -> bool:
    try:
        import jax

        return sum(1 for d in jax.devices() if d.platform != "cpu") >= N_CORES
    except Exception:
        return False


def _run_in_subprocess(x: np.ndarray) -> np.ndarray:
    with tempfile.TemporaryDirectory() as td:
        xin = os.path.join(td, "x.npy")
        xout = os.path.join(td, "y.npy")
        np.save(xin, x)
        env = dict(os.environ)
        for k in ("JAX_PLATFORMS", "JAX_PLATFORM_NAME"):
            env.pop(k, None)
        subprocess.run(
            [sys.executable, os.path.abspath(__file__), xin, xout],
            check=True,
            env=env,
        )
        return np.load(xout)


def kernel(x, children=None, child_mask=None, parents=None, parent_mask=None,
           topo=None, **_unused):
    x = np.ascontiguousarray(np.asarray(x), dtype=np.float32)
    if _trn_devices_visible():
        out, _ = _run(x)
        return out
    return _run_in_subprocess(x)


if __name__ == "__main__":
    _x = np.load(sys.argv[1])
    _out, _ = _run(_x)
    np.save(sys.argv[2], _out)
